# revision 3
# baseline (speedup 1.0000x reference)
"""Mamba block kernel v2 — instruction-count-minimized for the axon
dispatch-bound regime.

Sharding: core c = (batch c//2, d_inner half c%2), DL=1024 per core.
Pair AllReduce for x_proj partials, pair ReduceScatter for out_proj.

Structure per rep: three For_i hardware loops (P1 over j, scan over j,
out_proj over m) with DMA-staged stationary weights (ldweights needs
static addresses), big folded access patterns everywhere, and a single
tensor_tensor_scan per (j, n-half, chunk) with a[:, :, 0]=0 carry-kill.
"""
import sys
sys.path.insert(0, "/opt/trn_rl_repo")
import numpy as np
import ml_dtypes
import concourse.bass as bass
import concourse.bacc as bacc
import concourse.mybir as mybir
from concourse.tile import TileContext
from concourse.bass_utils import run_bass_kernel_spmd
from concourse.bass import ds

F32 = mybir.dt.float32
BF16 = mybir.dt.bfloat16
OP = mybir.AluOpType
AF = mybir.ActivationFunctionType
_SILU = AF.Silu

B_, L, DM = 4, 2048, 1024
DI = 2048
DL = 1024
N = 16
RK = 64
KC = 4
NJ = DL // 128          # 8
NK = DM // 128          # 8
NM = DM // 128          # 8
TC = 512
NCH = L // TC           # 4
PAIRS = [[0, 1], [2, 3], [4, 5], [6, 7]]

_CACHED_NC = {}


def _build(reps=1, variant="full", debug=False):
    nc = bacc.Bacc(num_devices=8)

    hst = nc.declare_dram_parameter("hst", [DM, L], BF16, isOutput=False)
    wpk = nc.declare_dram_parameter("wpk", [NJ, 128, 2 * NK * 128], BF16,
                                    isOutput=False)
    convw = nc.declare_dram_parameter("convw", [128, NJ, KC], F32,
                                      isOutput=False)
    convb = nc.declare_dram_parameter("convb", [128, NJ], F32, isOutput=False)
    wxp = nc.declare_dram_parameter("wxp", [128, NJ, RK + 2 * N], BF16,
                                    isOutput=False)
    wdtp = nc.declare_dram_parameter("wdtp", [NJ, RK, 128], BF16,
                                     isOutput=False)
    bdt = nc.declare_dram_parameter("bdt", [128, NJ], F32, isOutput=False)
    wop = nc.declare_dram_parameter("wop", [NM, 128, DL], BF16, isOutput=False)
    negAd = nc.declare_dram_parameter("negAd", [128, N, NJ], F32,
                                      isOutput=False)
    dvecd = nc.declare_dram_parameter("dvecd", [128, NJ], F32, isOutput=False)
    oslab = nc.declare_dram_parameter("oslab", [DM // 2, L], F32, isOutput=True)

    P = dict(hst=hst, wpk=wpk, convw=convw, convb=convb, wxp=wxp, wdtp=wdtp,
             bdt=bdt, wop=wop, negAd=negAd, dvecd=dvecd, oslab=oslab)
    if debug:
        for nm, shp, dt_ in [("dbg_u", [128, NJ, L], BF16),
                             ("dbg_g", [128, NJ, L], BF16),
                             ("dbg_yg", [128, NJ, L], BF16),
                             ("dbg_xdbl", [RK + 2 * N, L], F32),
                             ("dbg_dt", [128, L], F32),
                             ("dbg_bct", [128, 2 * N, TC], BF16),
                             ("dbg_xc", [128, KC - 1 + L], BF16),
                             ("dbg_acc", [128, L], BF16),
                             ("dbg_us", [128, L], BF16)]:
            P[nm] = nc.declare_dram_parameter(nm, shp, dt_, isOutput=True)
    P["debug"] = debug

    with TileContext(nc) as tc:
        with tc.tile_pool(name="const", bufs=1) as cp:
            C = {}
            C["convw"] = cp.tile([128, NJ, KC], F32, tag="cw", name="cw")
            nc.sync.dma_start(out=C["convw"][:, :, :], in_=convw[:, :, :])
            C["convb"] = cp.tile([128, NJ], F32, tag="cb", name="cb")
            nc.sync.dma_start(out=C["convb"][:, :], in_=convb[:, :])
            C["bdt"] = cp.tile([128, NJ], F32, tag="bd", name="bd")
            nc.sync.dma_start(out=C["bdt"][:, :], in_=bdt[:, :])
            C["negA"] = cp.tile([128, N, NJ], F32, tag="na", name="na")
            nc.sync.dma_start(out=C["negA"][:, :, :], in_=negAd[:, :, :])
            C["dvec"] = cp.tile([128, NJ], F32, tag="dv", name="dv")
            nc.sync.dma_start(out=C["dvec"][:, :], in_=dvecd[:, :])
            C["wxp"] = cp.tile([128, NJ, RK + 2 * N], BF16, tag="wx", name="wx")
            nc.sync.dma_start(out=C["wxp"][:, :, :], in_=wxp[:, :, :])

            for rep in range(reps):
                D_ = {}
                D_["xdbl_in"] = nc.dram_tensor(f"xdbl_in{rep}", [RK + 2 * N, L],
                                               F32)
                D_["xdbl_out"] = nc.dram_tensor(f"xdbl_out{rep}",
                                                [RK + 2 * N, L], F32)
                D_["xdbl_bf"] = nc.dram_tensor(f"xdbl_bf{rep}", [2 * N, L],
                                               BF16)
                D_["oc_in"] = nc.dram_tensor(f"oc_in{rep}", [NM, 128, L], F32)
                D_["oc_out"] = nc.dram_tensor(f"oc_out{rep}", [DM // 2, L], F32)
                _emit_rep(nc, tc, P, C, D_, variant)

    nc.finalize()
    return nc


def _emit_rep(nc, tc, P, C, D_, variant):
    nocoll = variant == "nocoll"
    with tc.tile_pool(name="res", bufs=1) as rp:
        u = rp.tile([128, NJ, L], BF16, tag="u", name="u")
        g = rp.tile([128, NJ, L], BF16, tag="g", name="g")
        yg = rp.tile([128, NJ, L], BF16, tag="yg", name="yg")
        nc.vector.memset(u[:, :, :], 0.0)
        nc.vector.memset(g[:, :, :], 0.0)
        nc.vector.memset(yg[:, :, :], 0.0)

        # ---------------- phase 1: in_proj x/z, conv, x_proj ----------------
        with (
            tc.tile_pool(name="p1", bufs=1) as p1,
            tc.tile_pool(name="ps1", bufs=1, space="PSUM") as ps1,
        ):
            hsT = p1.tile([128, NK, L], BF16, tag="hsT", name="hsT")
            nc.sync.dma_start(
                out=hsT[:, :, :],
                in_=P["hst"][:, :].rearrange("(k p) t -> p k t", k=NK))
            wslot = p1.tile([128, 2 * NK * 128], BF16, tag="wsl", name="wsl")
            wxpslot = p1.tile([128, RK + 2 * N], BF16, tag="wxs", name="wxs")
            cwslot = p1.tile([128, KC], F32, tag="cws", name="cws")
            cbslot = p1.tile([128, 1], F32, tag="cbs", name="cbs")
            xc = p1.tile([128, KC - 1 + L], BF16, tag="xc", name="xc")
            nc.vector.memset(xc[:, 0:KC - 1], 0.0)
            accA = p1.tile([128, L], BF16, tag="acA", name="acA")
            accB = p1.tile([128, L], BF16, tag="acB", name="acB")
            us = p1.tile([128, L], BF16, tag="us", name="us")
            zt = p1.tile([128, L], BF16, tag="zt", name="zt")
            xpacc = p1.tile([RK + 2 * N, L], F32, tag="xpa", name="xpa")
            nc.vector.memset(xpacc[:, :], 0.0)
            scr = p1.tile([RK + 2 * N, L], F32, tag="scr", name="scr")
            psA = ps1.tile([128, L], F32, tag="psA", name="psA")
            psB = ps1.tile([128, L], F32, tag="psB", name="psB")

            with tc.For_i(0, NJ) as j:
                nc.sync.dma_start(out=wslot[:, :], in_=P["wpk"][ds(j, 1), :, :])
                # x GEMM: k-outer, chunk-inner
                for k in range(NK):
                    for c in range(NCH):
                        csl = slice(c * TC, (c + 1) * TC)
                        nc.tensor.matmul(
                            psA[:, csl], wslot[:, k * 128:(k + 1) * 128],
                            hsT[:, k, csl], start=(k == 0), stop=(k == NK - 1))
                nc.vector.tensor_scalar(xc[:, KC - 1:KC - 1 + L], psA[:, :],
                                        0.0, 1.0, op0=OP.max, op1=OP.min)
                # depthwise conv: 4 shifted scalar mult-adds
                nc.vector.tensor_copy(cwslot[:, :],
                                      C["convw"][:, ds(j, 1), :].squeeze())
                nc.vector.tensor_scalar(accA[:, :], xc[:, 0:L],
                                        cwslot[:, 0:1], None, op0=OP.mult)
                nc.vector.scalar_tensor_tensor(
                    out=accB[:, :], in0=xc[:, 1:1 + L], scalar=cwslot[:, 1:2],
                    in1=accA[:, :], op0=OP.mult, op1=OP.add)
                nc.vector.scalar_tensor_tensor(
                    out=accA[:, :], in0=xc[:, 2:2 + L], scalar=cwslot[:, 2:3],
                    in1=accB[:, :], op0=OP.mult, op1=OP.add)
                nc.vector.scalar_tensor_tensor(
                    out=accB[:, :], in0=xc[:, 3:3 + L], scalar=cwslot[:, 3:4],
                    in1=accA[:, :], op0=OP.mult, op1=OP.add)
                nc.vector.tensor_copy(cbslot[:, 0:1], C["convb"][:, ds(j, 1)])
                nc.scalar.activation(us[:, :], accB[:, :], _SILU,
                                     bias=cbslot[:, 0:1])
                nc.vector.tensor_scalar(u[:, ds(j, 1), :].squeeze(), us[:, :],
                                        0.0, 1.0, op0=OP.max, op1=OP.min)
                # z GEMM
                for k in range(NK):
                    for c in range(NCH):
                        csl = slice(c * TC, (c + 1) * TC)
                        nc.tensor.matmul(
                            psB[:, csl],
                            wslot[:, NK * 128 + k * 128:NK * 128 + (k + 1) * 128],
                            hsT[:, k, csl], start=(k == 0), stop=(k == NK - 1))
                nc.vector.tensor_scalar(zt[:, :], psB[:, :], 0.0, 1.0,
                                        op0=OP.max, op1=OP.min)
                nc.scalar.activation(g[:, ds(j, 1), :].squeeze(), zt[:, :],
                                     _SILU)
                # x_proj partial: xpacc += wxp_j.T @ u_j
                nc.vector.tensor_copy(wxpslot[:, :],
                                      C["wxp"][:, ds(j, 1), :].squeeze())
                for c in range(NCH):
                    csl = slice(c * TC, (c + 1) * TC)
                    nc.tensor.matmul(psA[0:RK + 2 * N, csl], wxpslot[:, :],
                                     u[:, ds(j, 1), csl].squeeze(),
                                     start=True, stop=True)
                nc.scalar.copy(scr[:, :], psA[0:RK + 2 * N, :])
                nc.vector.tensor_tensor(out=xpacc[:, :], in0=xpacc[:, :],
                                        in1=scr[:, :], op=OP.add)

            nc.sync.dma_start(out=D_["xdbl_in"][:, :], in_=xpacc[:, :])
            if P["debug"]:
                nc.sync.dma_start(out=P["dbg_u"][:, :, :], in_=u[:, :, :])
                nc.sync.dma_start(out=P["dbg_g"][:, :, :], in_=g[:, :, :])
                nc.sync.dma_start(out=P["dbg_xc"][:, :], in_=xc[:, :])
                nc.sync.dma_start(out=P["dbg_acc"][:, :], in_=accB[:, :])
                nc.sync.dma_start(out=P["dbg_us"][:, :], in_=us[:, :])

        if nocoll:
            nc.gpsimd.dma_start(out=D_["xdbl_out"][:, :],
                                in_=D_["xdbl_in"][:, :])
        else:
            nc.gpsimd.collective_compute(
                "AllReduce", OP.add, replica_groups=PAIRS,
                ins=[D_["xdbl_in"][:, :]], outs=[D_["xdbl_out"][:, :]])

        # ---------------- scan phase ----------------
        with (
            tc.tile_pool(name="sc", bufs=1) as sp,
            tc.tile_pool(name="ps2", bufs=1, space="PSUM") as ps2,
        ):
            dtraw = sp.tile([RK, L], BF16, tag="dtr", name="dtr")
            wdtslot = sp.tile([RK, 128], BF16, tag="wds", name="wds")
            spe = sp.tile([128, L], F32, tag="spe", name="spe")
            dt = spe  # ln/clip run in place
            dtu = sp.tile([128, L], BF16, tag="dtu", name="dtu")

            # stage xdbl through spe/dtu slices (scratch reuse, saves a pool)
            nc.sync.dma_start(out=spe[0:RK + 2 * N, :], in_=D_["xdbl_out"][:, :])
            nc.vector.tensor_scalar(dtraw[:, :], spe[0:RK, :], 0.0, 1.0,
                                    op0=OP.max, op1=OP.min)
            nc.vector.tensor_copy(dtu[0:2 * N, :], spe[RK:RK + 2 * N, :])
            nc.sync.dma_start(out=D_["xdbl_bf"][:, :], in_=dtu[0:2 * N, :])
            if P["debug"]:
                nc.sync.dma_start(out=P["dbg_xdbl"][:, :],
                                  in_=spe[0:RK + 2 * N, :])
            dslot = sp.tile([128, 1], F32, tag="dsl", name="dsl")
            bdslot = sp.tile([128, 1], F32, tag="bds", name="bds")
            bct = sp.tile([128, 2 * N, TC], BF16, tag="bct", name="bct")
            an = sp.tile([128, N, TC], BF16, tag="an", name="an")
            bt = sp.tile([128, N, TC], BF16, tag="bt", name="bt")
            ht = sp.tile([128, N, TC], BF16, tag="ht", name="ht")
            carry = sp.tile([128, N], F32, tag="car", name="car")
            tmpc = sp.tile([128, N], BF16, tag="tmc", name="tmc")
            yab = sp.tile([128, TC], F32, tag="yab", name="yab")
            ytmp = sp.tile([128, TC], F32, tag="ytm", name="ytm")
            ytc = sp.tile([128, TC], BF16, tag="ytc", name="ytc")
            psD = ps2.tile([128, L], F32, tag="psD", name="psD")

            with tc.For_i(0, NJ) as j:
                nc.sync.dma_start(out=wdtslot[:, :],
                                  in_=P["wdtp"][ds(j, 1), :, :])
                for c in range(NCH):
                    csl = slice(c * TC, (c + 1) * TC)
                    nc.tensor.matmul(psD[:, csl], wdtslot[:, :],
                                     dtraw[:, csl], start=True, stop=True)
                nc.vector.tensor_copy(bdslot[:, 0:1], C["bdt"][:, ds(j, 1)])
                nc.scalar.activation(spe[:, :], psD[:, :], AF.Exp,
                                     bias=bdslot[:, 0:1])
                nc.scalar.activation(dt[:, :], spe[:, :], AF.Ln, bias=1.0)
                nc.vector.tensor_scalar(dt[:, :], dt[:, :], 1e-4, 20.0,
                                        op0=OP.max, op1=OP.min)
                nc.vector.tensor_tensor(out=dtu[:, :], in0=dt[:, :],
                                        in1=u[:, ds(j, 1), :].squeeze(),
                                        op=OP.mult)
                nc.vector.tensor_copy(dslot[:, 0:1], C["dvec"][:, ds(j, 1)])
                for c in range(NCH):
                    csl = slice(c * TC, (c + 1) * TC)
                    nc.sync.dma_start(
                        out=bct[:, :, :],
                        in_=D_["xdbl_bf"][None, :, csl].broadcast_to(
                            [128, 2 * N, TC]))
                    nc.vector.tensor_tensor(
                        out=an[:, :, :],
                        in0=dt[:, None, csl].broadcast_to([128, N, TC]),
                        in1=C["negA"][:, :, ds(j, 1)].broadcast_to(
                            [128, N, TC]),
                        op=OP.mult)
                    nc.scalar.activation(an[:, :, :], an[:, :, :], AF.Exp)
                    nc.vector.tensor_tensor(
                        out=bt[:, :, :],
                        in0=dtu[:, None, csl].broadcast_to([128, N, TC]),
                        in1=bct[:, 0:N, :], op=OP.mult)
                    if c > 0:
                        nc.vector.tensor_tensor(
                            out=tmpc[:, :], in0=an[:, :, 0],
                            in1=carry[:, :], op=OP.mult)
                        nc.vector.tensor_tensor(
                            out=bt[:, :, 0], in0=bt[:, :, 0],
                            in1=tmpc[:, :], op=OP.add)
                    nc.vector.memset(an[:, :, 0], 0.0)
                    nc.vector.tensor_tensor_scan(
                        ht[:, :, :].rearrange("p n t -> p (n t)"),
                        an[:, :, :].rearrange("p n t -> p (n t)"),
                        bt[:, :, :].rearrange("p n t -> p (n t)"),
                        0.0, op0=OP.mult, op1=OP.add)
                    if c < NCH - 1:
                        nc.vector.tensor_copy(carry[:, :], ht[:, :, TC - 1])
                    # ch reuses bt's memory (bt is dead after the scan)
                    nc.vector.tensor_tensor(
                        out=bt[:, :, :], in0=ht[:, :, :],
                        in1=bct[:, N:2 * N, :], op=OP.mult)
                    nc.vector.reduce_sum(
                        yab[:, :], bt[:, :, :].rearrange("p n t -> p t n"),
                        axis=mybir.AxisListType.X)
                    nc.vector.scalar_tensor_tensor(
                        out=ytmp[:, :], in0=u[:, ds(j, 1), csl].squeeze(),
                        scalar=dslot[:, 0:1], in1=yab[:, :],
                        op0=OP.mult, op1=OP.add)
                    nc.vector.tensor_scalar(ytc[:, :], ytmp[:, :], 0.0, 1.0,
                                            op0=OP.max, op1=OP.min)
                    nc.vector.tensor_tensor(
                        out=yg[:, ds(j, 1), csl].squeeze(), in0=ytc[:, :],
                        in1=g[:, ds(j, 1), csl].squeeze(), op=OP.mult)
            if P["debug"]:
                nc.sync.dma_start(out=P["dbg_dt"][:, :], in_=dt[:, :])
                nc.sync.dma_start(out=P["dbg_bct"][:, :, :], in_=bct[:, :, :])
                nc.sync.dma_start(out=P["dbg_yg"][:, :, :], in_=yg[:, :, :])

        # ---------------- out_proj ----------------
        with (
            tc.tile_pool(name="op", bufs=1) as op_,
            tc.tile_pool(name="ps3", bufs=1, space="PSUM") as ps3,
        ):
            woslot = op_.tile([128, DL], BF16, tag="wos", name="wos")
            ocs = op_.tile([128, L], F32, tag="ocs", name="ocs")
            psO = ps3.tile([128, L], F32, tag="psO", name="psO")
            with tc.For_i(0, NM) as m:
                nc.sync.dma_start(out=woslot[:, :],
                                  in_=P["wop"][ds(m, 1), :, :])
                for jj in range(NJ):
                    for c in range(NCH):
                        csl = slice(c * TC, (c + 1) * TC)
                        nc.tensor.matmul(
                            psO[:, csl], woslot[:, jj * 128:(jj + 1) * 128],
                            yg[:, jj, csl], start=(jj == 0),
                            stop=(jj == NJ - 1))
                nc.scalar.copy(ocs[:, :], psO[:, :])
                nc.sync.dma_start(out=D_["oc_in"][ds(m, 1), :, :],
                                  in_=ocs[:, :])
        if nocoll:
            nc.gpsimd.dma_start(out=D_["oc_out"][:, :],
                                in_=D_["oc_in"][0:NM // 2, :, :])
        else:
            nc.gpsimd.collective_compute(
                "ReduceScatter", OP.add, replica_groups=PAIRS,
                ins=[D_["oc_in"][:, :, :]], outs=[D_["oc_out"][:, :]])
        nc.gpsimd.dma_start(out=P["oslab"][:, :], in_=D_["oc_out"][:, :])


def _shard(inputs):
    hs = np.asarray(inputs["hidden_states"], np.float32)
    W_in = np.asarray(inputs["W_in"], np.float32)
    conv_w = np.asarray(inputs["conv_w"], np.float32)
    conv_b = np.asarray(inputs["conv_b"], np.float32)
    W_x = np.asarray(inputs["W_x"], np.float32)
    W_dt = np.asarray(inputs["W_dt"], np.float32)
    b_dt = np.asarray(inputs["b_dt"], np.float32)
    W_out = np.asarray(inputs["W_out"], np.float32)
    A_log = np.asarray(inputs["A_log"], np.float32)
    Dv = np.asarray(inputs["D"], np.float32)
    bf = ml_dtypes.bfloat16

    in_maps = []
    for c in range(8):
        b, dh = c // 2, c % 2
        dsl = slice(dh * DL, (dh + 1) * DL)
        # wpk[j, p, k*128+q] = W_in[dh*DL + j*128 + q, k*128 + p]  (x)
        # wpk[j, p, 1024 + k*128+q] = W_in[DI + ..., k*128 + p]    (z)
        Wx = W_in[dsl].reshape(NJ, 128, NK, 128)       # [j, q, k, p]
        Wz = W_in[DI + dh * DL:DI + (dh + 1) * DL].reshape(NJ, 128, NK, 128)
        wpk = np.concatenate(
            [Wx.transpose(0, 3, 2, 1).reshape(NJ, 128, NK * 128),
             Wz.transpose(0, 3, 2, 1).reshape(NJ, 128, NK * 128)],
            axis=2).astype(bf)
        # wpk[j, p, k*128+q]: index p is within k-tile; transpose(0,3,2,1)
        # gives [j, p, k, q] -> reshape [j, p, k*128+q]. OK.
        convw_l = conv_w[dsl, 0, :]                    # (DL, KC)
        convw_m = convw_l.reshape(NJ, 128, KC).transpose(1, 0, 2)  # [p, j, k]
        convb_m = conv_b[dsl].reshape(NJ, 128).T
        # wxp[p, j, r] = W_x[r, dh*DL + j*128 + p]
        wxp_m = W_x[:, dsl].reshape(RK + 2 * N, NJ, 128).transpose(
            2, 1, 0).astype(bf)
        # wdtp[j, r, q] = W_dt[dh*DL + j*128 + q, r]
        wdt_m = W_dt[dsl].reshape(NJ, 128, RK).transpose(0, 2, 1).astype(bf)
        bdt_m = b_dt[dsl].reshape(NJ, 128).T
        # wop[m, p, j*128+q] = W_out[m*128+q, dh*DL + j*128 + p]
        wop_m = W_out[:, dsl].reshape(NM, 128, NJ, 128).transpose(
            0, 3, 2, 1).reshape(NM, 128, NJ * 128).astype(bf)
        # negA[p, n, j] = -exp(A_log[dh*DL + j*128 + p, n])
        negA_m = -np.exp(A_log[dsl]).reshape(NJ, 128, N).transpose(1, 2, 0)
        dvec_m = Dv[dsl].reshape(NJ, 128).T
        m = {
            "hst": np.ascontiguousarray(hs[b].T).astype(bf),
            "wpk": np.ascontiguousarray(wpk),
            "convw": np.ascontiguousarray(convw_m),
            "convb": np.ascontiguousarray(convb_m),
            "wxp": np.ascontiguousarray(wxp_m),
            "wdtp": np.ascontiguousarray(wdt_m),
            "bdt": np.ascontiguousarray(bdt_m),
            "wop": np.ascontiguousarray(wop_m),
            "negAd": np.ascontiguousarray(negA_m),
            "dvecd": np.ascontiguousarray(dvec_m),
        }
        in_maps.append(m)
    return in_maps


def kernel(**inputs):
    if 1 not in _CACHED_NC:
        _CACHED_NC[1] = _build(1)
    nc = _CACHED_NC[1]
    in_maps = _shard(inputs)
    res = run_bass_kernel_spmd(nc, in_maps, core_ids=list(range(8)))
    out = np.empty((B_, L, DM), np.float32)
    for b in range(B_):
        s0 = res.results[2 * b]["oslab"]
        s1 = res.results[2 * b + 1]["oslab"]
        out[b] = np.concatenate([s0, s1], axis=0).T
    return out


# revision 4
# speedup vs baseline: 1.3247x; 1.3247x over previous
"""Mamba block kernel v2 — instruction-count-minimized for the axon
dispatch-bound regime.

Sharding: core c = (batch c//2, d_inner half c%2), DL=1024 per core.
Pair AllReduce for x_proj partials, pair ReduceScatter for out_proj.

Structure per rep: three For_i hardware loops (P1 over j, scan over j,
out_proj over m) with DMA-staged stationary weights (ldweights needs
static addresses), big folded access patterns everywhere, and a single
tensor_tensor_scan per (j, n-half, chunk) with a[:, :, 0]=0 carry-kill.
"""
import sys
sys.path.insert(0, "/opt/trn_rl_repo")
import numpy as np
import ml_dtypes
import concourse.bass as bass
import concourse.bacc as bacc
import concourse.mybir as mybir
from concourse.tile import TileContext
from concourse.bass_utils import run_bass_kernel_spmd
from concourse.bass import ds

F32 = mybir.dt.float32
BF16 = mybir.dt.bfloat16
OP = mybir.AluOpType
AF = mybir.ActivationFunctionType
_SILU = AF.Silu

B_, L, DM = 4, 2048, 1024
DI = 2048
DL = 1024
N = 16
RK = 64
KC = 4
NJ = DL // 128          # 8
NK = DM // 128          # 8
NM = DM // 128          # 8
TC = 512
NCH = L // TC           # 4
PAIRS = [[0, 1], [2, 3], [4, 5], [6, 7]]

_CACHED_NC = {}


def _build(reps=1, variant="full", debug=False):
    nc = bacc.Bacc(num_devices=8)

    hst = nc.declare_dram_parameter("hst", [DM, L], BF16, isOutput=False)
    wpk = nc.declare_dram_parameter("wpk", [NJ, 128, 2 * NK * 128], BF16,
                                    isOutput=False)
    convw = nc.declare_dram_parameter("convw", [128, NJ, KC], F32,
                                      isOutput=False)
    convb = nc.declare_dram_parameter("convb", [128, NJ], F32, isOutput=False)
    wxp = nc.declare_dram_parameter("wxp", [128, NJ, RK + 2 * N], BF16,
                                    isOutput=False)
    wdtp = nc.declare_dram_parameter("wdtp", [NJ, RK, 128], BF16,
                                     isOutput=False)
    bdt = nc.declare_dram_parameter("bdt", [128, NJ], F32, isOutput=False)
    wop = nc.declare_dram_parameter("wop", [NM, 128, DL], BF16, isOutput=False)
    negAd = nc.declare_dram_parameter("negAd", [128, N, NJ], F32,
                                      isOutput=False)
    dvecd = nc.declare_dram_parameter("dvecd", [128, NJ], F32, isOutput=False)
    oslab = nc.declare_dram_parameter("oslab", [DM // 2, L], F32, isOutput=True)

    P = dict(hst=hst, wpk=wpk, convw=convw, convb=convb, wxp=wxp, wdtp=wdtp,
             bdt=bdt, wop=wop, negAd=negAd, dvecd=dvecd, oslab=oslab)
    if debug:
        for nm, shp, dt_ in [("dbg_u", [128, NJ, L], BF16),
                             ("dbg_g", [128, NJ, L], BF16),
                             ("dbg_yg", [128, NJ, L], BF16),
                             ("dbg_xdbl", [RK + 2 * N, L], F32),
                             ("dbg_dt", [128, L], F32),
                             ("dbg_bct", [128, 2 * N, TC], BF16),
                             ("dbg_xc", [128, KC - 1 + L], BF16),
                             ("dbg_acc", [128, L], BF16),
                             ("dbg_us", [128, L], BF16)]:
            P[nm] = nc.declare_dram_parameter(nm, shp, dt_, isOutput=True)
    P["debug"] = debug

    with TileContext(nc) as tc:
        with tc.tile_pool(name="const", bufs=1) as cp:
            C = {}
            C["convw"] = cp.tile([128, NJ, KC], F32, tag="cw", name="cw")
            nc.sync.dma_start(out=C["convw"][:, :, :], in_=convw[:, :, :])
            C["convb"] = cp.tile([128, NJ], F32, tag="cb", name="cb")
            nc.sync.dma_start(out=C["convb"][:, :], in_=convb[:, :])
            C["bdt"] = cp.tile([128, NJ], F32, tag="bd", name="bd")
            nc.sync.dma_start(out=C["bdt"][:, :], in_=bdt[:, :])
            C["negA"] = cp.tile([128, N, NJ], F32, tag="na", name="na")
            nc.sync.dma_start(out=C["negA"][:, :, :], in_=negAd[:, :, :])
            C["dvec"] = cp.tile([128, NJ], F32, tag="dv", name="dv")
            nc.sync.dma_start(out=C["dvec"][:, :], in_=dvecd[:, :])
            C["wxp"] = cp.tile([128, NJ, RK + 2 * N], BF16, tag="wx", name="wx")
            nc.sync.dma_start(out=C["wxp"][:, :, :], in_=wxp[:, :, :])
            C["wdt"] = cp.tile([RK, NJ, 128], BF16, tag="wdtr", name="wdtr")
            nc.sync.dma_start(out=C["wdt"][:, :, :],
                              in_=wdtp[:, :, :].rearrange("j r q -> r j q"))


            for rep in range(reps):
                D_ = {}
                D_["xdbl_in"] = nc.dram_tensor(f"xdbl_in{rep}", [RK + 2 * N, L],
                                               F32)
                D_["xdbl_out"] = nc.dram_tensor(f"xdbl_out{rep}",
                                                [RK + 2 * N, L], F32)
                D_["xdbl_bf"] = nc.dram_tensor(f"xdbl_bf{rep}", [2 * N, L],
                                               BF16)
                D_["oc_in"] = nc.dram_tensor(f"oc_in{rep}", [NM, 128, L], F32)
                D_["oc_out"] = nc.dram_tensor(f"oc_out{rep}", [DM // 2, L], F32)
                _emit_rep(nc, tc, P, C, D_, variant)

    nc.finalize()
    return nc


def _emit_rep(nc, tc, P, C, D_, variant):
    nocoll = variant == "nocoll"
    with tc.tile_pool(name="res", bufs=1) as rp:
        u = rp.tile([128, NJ, L], BF16, tag="u", name="u")
        g = rp.tile([128, NJ, L], BF16, tag="g", name="g")
        yg = rp.tile([128, NJ, L], BF16, tag="yg", name="yg")
        nc.vector.memset(u[:, :, :], 0.0)
        nc.vector.memset(g[:, :, :], 0.0)
        nc.vector.memset(yg[:, :, :], 0.0)

        # ---------------- phase 1: in_proj x/z, conv, x_proj ----------------
        with (
            tc.tile_pool(name="p1", bufs=1) as p1,
            tc.tile_pool(name="ps1", bufs=1, space="PSUM") as ps1,
        ):
            hsT = p1.tile([128, NK, L], BF16, tag="hsT", name="hsT")
            nc.sync.dma_start(
                out=hsT[:, :, :],
                in_=P["hst"][:, :].rearrange("(k p) t -> p k t", k=NK))
            wpkr = p1.tile([128, NJ, 2 * NK * 128], BF16, tag="wpk",
                           name="wpkr")
            nc.sync.dma_start(out=wpkr[:, :, :],
                              in_=P["wpk"][:, :, :].rearrange("j p w -> p j w"))
            wslot = p1.tile([128, 2 * NK * 128], BF16, tag="wsl", name="wsl")
            wxpslot = p1.tile([128, RK + 2 * N], BF16, tag="wxs", name="wxs")
            cwslot = p1.tile([128, KC], F32, tag="cws", name="cws")
            cbslot = p1.tile([128, 1], F32, tag="cbs", name="cbs")
            xc = p1.tile([128, KC - 1 + L], BF16, tag="xc", name="xc")
            nc.vector.memset(xc[:, 0:KC - 1], 0.0)
            accA = p1.tile([128, L], BF16, tag="acA", name="acA")
            accB = p1.tile([128, L], BF16, tag="acB", name="acB")
            xpacc = p1.tile([RK + 2 * N, L], F32, tag="xpa", name="xpa")
            nc.vector.memset(xpacc[:, :], 0.0)
            psA = ps1.tile([128, L], F32, tag="psA", name="psA")
            psB = ps1.tile([128, L], F32, tag="psB", name="psB")

            with tc.For_i(0, NJ) as j:
                nc.vector.tensor_copy(wslot[:, :],
                                      wpkr[:, ds(j, 1), :].squeeze())
                # x GEMM: k-outer, chunk-inner
                for k in range(NK):
                    for c in range(NCH):
                        csl = slice(c * TC, (c + 1) * TC)
                        nc.tensor.matmul(
                            psA[:, csl], wslot[:, k * 128:(k + 1) * 128],
                            hsT[:, k, csl], start=(k == 0), stop=(k == NK - 1))
                nc.vector.tensor_scalar(xc[:, KC - 1:KC - 1 + L], psA[:, :],
                                        0.0, 1.0, op0=OP.max, op1=OP.min)
                # depthwise conv: 4 shifted scalar mult-adds
                nc.vector.tensor_copy(cwslot[:, :],
                                      C["convw"][:, ds(j, 1), :].squeeze())
                nc.vector.tensor_scalar(accA[:, :], xc[:, 0:L],
                                        cwslot[:, 0:1], None, op0=OP.mult)
                nc.vector.scalar_tensor_tensor(
                    out=accB[:, :], in0=xc[:, 1:1 + L], scalar=cwslot[:, 1:2],
                    in1=accA[:, :], op0=OP.mult, op1=OP.add)
                nc.vector.scalar_tensor_tensor(
                    out=accA[:, :], in0=xc[:, 2:2 + L], scalar=cwslot[:, 2:3],
                    in1=accB[:, :], op0=OP.mult, op1=OP.add)
                nc.vector.scalar_tensor_tensor(
                    out=accB[:, :], in0=xc[:, 3:3 + L], scalar=cwslot[:, 3:4],
                    in1=accA[:, :], op0=OP.mult, op1=OP.add)
                nc.vector.tensor_copy(cbslot[:, 0:1], C["convb"][:, ds(j, 1)])
                nc.scalar.activation(accA[:, :], accB[:, :], _SILU,
                                     bias=cbslot[:, 0:1])
                nc.vector.tensor_scalar(u[:, ds(j, 1), :].squeeze(),
                                        accA[:, :],
                                        0.0, 1.0, op0=OP.max, op1=OP.min)
                # z GEMM
                for k in range(NK):
                    for c in range(NCH):
                        csl = slice(c * TC, (c + 1) * TC)
                        nc.tensor.matmul(
                            psB[:, csl],
                            wslot[:, NK * 128 + k * 128:NK * 128 + (k + 1) * 128],
                            hsT[:, k, csl], start=(k == 0), stop=(k == NK - 1))
                nc.vector.tensor_scalar(accB[:, :], psB[:, :], 0.0, 1.0,
                                        op0=OP.max, op1=OP.min)
                nc.scalar.activation(g[:, ds(j, 1), :].squeeze(), accB[:, :],
                                     _SILU)
                # x_proj partial: xpacc += wxp_j.T @ u_j
                nc.vector.tensor_copy(wxpslot[:, :],
                                      C["wxp"][:, ds(j, 1), :].squeeze())
                for c in range(NCH):
                    csl = slice(c * TC, (c + 1) * TC)
                    nc.tensor.matmul(psA[0:RK + 2 * N, csl], wxpslot[:, :],
                                     u[:, ds(j, 1), csl].squeeze(),
                                     start=True, stop=True)
                nc.vector.tensor_tensor(out=xpacc[:, :], in0=xpacc[:, :],
                                        in1=psA[0:RK + 2 * N, :], op=OP.add)

            nc.sync.dma_start(out=D_["xdbl_in"][:, :], in_=xpacc[:, :])
            if P["debug"]:
                nc.sync.dma_start(out=P["dbg_u"][:, :, :], in_=u[:, :, :])
                nc.sync.dma_start(out=P["dbg_g"][:, :, :], in_=g[:, :, :])
                nc.sync.dma_start(out=P["dbg_xc"][:, :], in_=xc[:, :])
                nc.sync.dma_start(out=P["dbg_acc"][:, :], in_=accB[:, :])
                nc.sync.dma_start(out=P["dbg_us"][:, :], in_=us[:, :])

        if nocoll:
            nc.gpsimd.dma_start(out=D_["xdbl_out"][:, :],
                                in_=D_["xdbl_in"][:, :])
        else:
            nc.gpsimd.collective_compute(
                "AllReduce", OP.add, replica_groups=PAIRS,
                ins=[D_["xdbl_in"][:, :]], outs=[D_["xdbl_out"][:, :]])

        # ---------------- scan phase ----------------
        with (
            tc.tile_pool(name="sc", bufs=1) as sp,
            tc.tile_pool(name="ps2", bufs=1, space="PSUM") as ps2,
        ):
            dtraw = sp.tile([RK, L], BF16, tag="dtr", name="dtr")
            wdtslot = sp.tile([RK, 128], BF16, tag="wds", name="wds")
            spe = sp.tile([128, L], F32, tag="spe", name="spe")
            dt = spe  # ln/clip run in place
            dtu = sp.tile([128, L], BF16, tag="dtu", name="dtu")

            # stage xdbl through spe/dtu slices (scratch reuse, saves a pool)
            nc.sync.dma_start(out=spe[0:RK + 2 * N, :], in_=D_["xdbl_out"][:, :])
            nc.vector.tensor_scalar(dtraw[:, :], spe[0:RK, :], 0.0, 1.0,
                                    op0=OP.max, op1=OP.min)
            nc.vector.tensor_copy(dtu[0:2 * N, :], spe[RK:RK + 2 * N, :])
            nc.sync.dma_start(out=D_["xdbl_bf"][:, :], in_=dtu[0:2 * N, :])
            if P["debug"]:
                nc.sync.dma_start(out=P["dbg_xdbl"][:, :],
                                  in_=spe[0:RK + 2 * N, :])
            dslot = sp.tile([128, 1], F32, tag="dsl", name="dsl")
            bdslot = sp.tile([128, 1], F32, tag="bds", name="bds")
            bct = sp.tile([128, 2 * N, TC], BF16, tag="bct", name="bct")
            an = sp.tile([128, N, TC], BF16, tag="an", name="an")
            bt = sp.tile([128, N, TC], BF16, tag="bt", name="bt")
            ht = sp.tile([128, N, TC], BF16, tag="ht", name="ht")
            carry = sp.tile([128, N], F32, tag="car", name="car")
            tmpc = sp.tile([128, N], BF16, tag="tmc", name="tmc")
            yab = sp.tile([128, TC], F32, tag="yab", name="yab")
            ytmp = sp.tile([128, TC], F32, tag="ytm", name="ytm")
            ytc = sp.tile([128, TC], BF16, tag="ytc", name="ytc")
            psD = ps2.tile([128, L], F32, tag="psD", name="psD")

            with tc.For_i(0, NJ) as j:
                nc.vector.tensor_copy(wdtslot[:, :],
                                      C["wdt"][:, ds(j, 1), :].squeeze())
                for c in range(NCH):
                    csl = slice(c * TC, (c + 1) * TC)
                    nc.tensor.matmul(psD[:, csl], wdtslot[:, :],
                                     dtraw[:, csl], start=True, stop=True)
                nc.vector.tensor_copy(bdslot[:, 0:1], C["bdt"][:, ds(j, 1)])
                nc.scalar.activation(spe[:, :], psD[:, :], AF.Exp,
                                     bias=bdslot[:, 0:1])
                nc.scalar.activation(dt[:, :], spe[:, :], AF.Ln, bias=1.0)
                nc.vector.tensor_scalar(dt[:, :], dt[:, :], 1e-4, 20.0,
                                        op0=OP.max, op1=OP.min)
                nc.vector.tensor_tensor(out=dtu[:, :], in0=dt[:, :],
                                        in1=u[:, ds(j, 1), :].squeeze(),
                                        op=OP.mult)
                nc.vector.tensor_copy(dslot[:, 0:1], C["dvec"][:, ds(j, 1)])
                for c in range(NCH):
                    csl = slice(c * TC, (c + 1) * TC)
                    nc.sync.dma_start(
                        out=bct[:, :, :],
                        in_=D_["xdbl_bf"][None, :, csl].broadcast_to(
                            [128, 2 * N, TC]))
                    nc.vector.tensor_tensor(
                        out=an[:, :, :],
                        in0=dt[:, None, csl].broadcast_to([128, N, TC]),
                        in1=C["negA"][:, :, ds(j, 1)].broadcast_to(
                            [128, N, TC]),
                        op=OP.mult)
                    nc.scalar.activation(an[:, :, :], an[:, :, :], AF.Exp)
                    nc.vector.tensor_tensor(
                        out=bt[:, :, :],
                        in0=dtu[:, None, csl].broadcast_to([128, N, TC]),
                        in1=bct[:, 0:N, :], op=OP.mult)
                    if c > 0:
                        nc.vector.tensor_tensor(
                            out=tmpc[:, :], in0=an[:, :, 0],
                            in1=carry[:, :], op=OP.mult)
                        nc.vector.tensor_tensor(
                            out=bt[:, :, 0], in0=bt[:, :, 0],
                            in1=tmpc[:, :], op=OP.add)
                    nc.vector.memset(an[:, :, 0], 0.0)
                    nc.vector.tensor_tensor_scan(
                        ht[:, :, :].rearrange("p n t -> p (n t)"),
                        an[:, :, :].rearrange("p n t -> p (n t)"),
                        bt[:, :, :].rearrange("p n t -> p (n t)"),
                        0.0, op0=OP.mult, op1=OP.add)
                    if c < NCH - 1:
                        nc.vector.tensor_copy(carry[:, :], ht[:, :, TC - 1])
                    # ch reuses bt's memory (bt is dead after the scan)
                    nc.vector.tensor_tensor(
                        out=bt[:, :, :], in0=ht[:, :, :],
                        in1=bct[:, N:2 * N, :], op=OP.mult)
                    nc.vector.reduce_sum(
                        yab[:, :], bt[:, :, :].rearrange("p n t -> p t n"),
                        axis=mybir.AxisListType.X)
                    nc.vector.scalar_tensor_tensor(
                        out=ytmp[:, :], in0=u[:, ds(j, 1), csl].squeeze(),
                        scalar=dslot[:, 0:1], in1=yab[:, :],
                        op0=OP.mult, op1=OP.add)
                    nc.vector.tensor_scalar(ytc[:, :], ytmp[:, :], 0.0, 1.0,
                                            op0=OP.max, op1=OP.min)
                    nc.vector.tensor_tensor(
                        out=yg[:, ds(j, 1), csl].squeeze(), in0=ytc[:, :],
                        in1=g[:, ds(j, 1), csl].squeeze(), op=OP.mult)
            if P["debug"]:
                nc.sync.dma_start(out=P["dbg_dt"][:, :], in_=dt[:, :])
                nc.sync.dma_start(out=P["dbg_bct"][:, :, :], in_=bct[:, :, :])
                nc.sync.dma_start(out=P["dbg_yg"][:, :, :], in_=yg[:, :, :])

        # ---------------- out_proj ----------------
        with (
            tc.tile_pool(name="op", bufs=1) as op_,
            tc.tile_pool(name="ps3", bufs=1, space="PSUM") as ps3,
        ):
            wor = op_.tile([128, NM, DL], BF16, tag="wor", name="wor")
            nc.sync.dma_start(out=wor[:, :, :],
                              in_=P["wop"][:, :, :].rearrange("m p w -> p m w"))
            woslot = op_.tile([128, DL], BF16, tag="wos", name="wos")
            ocs = op_.tile([128, L], F32, tag="ocs", name="ocs")
            psO = ps3.tile([128, L], F32, tag="psO", name="psO")
            with tc.For_i(0, NM) as m:
                nc.vector.tensor_copy(woslot[:, :],
                                      wor[:, ds(m, 1), :].squeeze())
                for jj in range(NJ):
                    for c in range(NCH):
                        csl = slice(c * TC, (c + 1) * TC)
                        nc.tensor.matmul(
                            psO[:, csl], woslot[:, jj * 128:(jj + 1) * 128],
                            yg[:, jj, csl], start=(jj == 0),
                            stop=(jj == NJ - 1))
                nc.scalar.copy(ocs[:, :], psO[:, :])
                nc.sync.dma_start(out=D_["oc_in"][ds(m, 1), :, :],
                                  in_=ocs[:, :])
        if nocoll:
            nc.gpsimd.dma_start(out=D_["oc_out"][:, :],
                                in_=D_["oc_in"][0:NM // 2, :, :])
        else:
            nc.gpsimd.collective_compute(
                "ReduceScatter", OP.add, replica_groups=PAIRS,
                ins=[D_["oc_in"][:, :, :]], outs=[D_["oc_out"][:, :]])
        nc.gpsimd.dma_start(out=P["oslab"][:, :], in_=D_["oc_out"][:, :])


def _shard(inputs):
    hs = np.asarray(inputs["hidden_states"], np.float32)
    W_in = np.asarray(inputs["W_in"], np.float32)
    conv_w = np.asarray(inputs["conv_w"], np.float32)
    conv_b = np.asarray(inputs["conv_b"], np.float32)
    W_x = np.asarray(inputs["W_x"], np.float32)
    W_dt = np.asarray(inputs["W_dt"], np.float32)
    b_dt = np.asarray(inputs["b_dt"], np.float32)
    W_out = np.asarray(inputs["W_out"], np.float32)
    A_log = np.asarray(inputs["A_log"], np.float32)
    Dv = np.asarray(inputs["D"], np.float32)
    bf = ml_dtypes.bfloat16

    in_maps = []
    for c in range(8):
        b, dh = c // 2, c % 2
        dsl = slice(dh * DL, (dh + 1) * DL)
        # wpk[j, p, k*128+q] = W_in[dh*DL + j*128 + q, k*128 + p]  (x)
        # wpk[j, p, 1024 + k*128+q] = W_in[DI + ..., k*128 + p]    (z)
        Wx = W_in[dsl].reshape(NJ, 128, NK, 128)       # [j, q, k, p]
        Wz = W_in[DI + dh * DL:DI + (dh + 1) * DL].reshape(NJ, 128, NK, 128)
        wpk = np.concatenate(
            [Wx.transpose(0, 3, 2, 1).reshape(NJ, 128, NK * 128),
             Wz.transpose(0, 3, 2, 1).reshape(NJ, 128, NK * 128)],
            axis=2).astype(bf)
        # wpk[j, p, k*128+q]: index p is within k-tile; transpose(0,3,2,1)
        # gives [j, p, k, q] -> reshape [j, p, k*128+q]. OK.
        convw_l = conv_w[dsl, 0, :]                    # (DL, KC)
        convw_m = convw_l.reshape(NJ, 128, KC).transpose(1, 0, 2)  # [p, j, k]
        convb_m = conv_b[dsl].reshape(NJ, 128).T
        # wxp[p, j, r] = W_x[r, dh*DL + j*128 + p]
        wxp_m = W_x[:, dsl].reshape(RK + 2 * N, NJ, 128).transpose(
            2, 1, 0).astype(bf)
        # wdtp[j, r, q] = W_dt[dh*DL + j*128 + q, r]
        wdt_m = W_dt[dsl].reshape(NJ, 128, RK).transpose(0, 2, 1).astype(bf)
        bdt_m = b_dt[dsl].reshape(NJ, 128).T
        # wop[m, p, j*128+q] = W_out[m*128+q, dh*DL + j*128 + p]
        wop_m = W_out[:, dsl].reshape(NM, 128, NJ, 128).transpose(
            0, 3, 2, 1).reshape(NM, 128, NJ * 128).astype(bf)
        # negA[p, n, j] = -exp(A_log[dh*DL + j*128 + p, n])
        negA_m = -np.exp(A_log[dsl]).reshape(NJ, 128, N).transpose(1, 2, 0)
        dvec_m = Dv[dsl].reshape(NJ, 128).T
        m = {
            "hst": np.ascontiguousarray(hs[b].T).astype(bf),
            "wpk": np.ascontiguousarray(wpk),
            "convw": np.ascontiguousarray(convw_m),
            "convb": np.ascontiguousarray(convb_m),
            "wxp": np.ascontiguousarray(wxp_m),
            "wdtp": np.ascontiguousarray(wdt_m),
            "bdt": np.ascontiguousarray(bdt_m),
            "wop": np.ascontiguousarray(wop_m),
            "negAd": np.ascontiguousarray(negA_m),
            "dvecd": np.ascontiguousarray(dvec_m),
        }
        in_maps.append(m)
    return in_maps


def kernel(**inputs):
    if 1 not in _CACHED_NC:
        _CACHED_NC[1] = _build(1)
    nc = _CACHED_NC[1]
    in_maps = _shard(inputs)
    res = run_bass_kernel_spmd(nc, in_maps, core_ids=list(range(8)))
    out = np.empty((B_, L, DM), np.float32)
    for b in range(B_):
        s0 = res.results[2 * b]["oslab"]
        s1 = res.results[2 * b + 1]["oslab"]
        out[b] = np.concatenate([s0, s1], axis=0).T
    return out
